# revision 18
# baseline (speedup 1.0000x reference)
"""Trainium2 Bass kernel: dense transformer attention block (QKV proj + RoPE +
GQA causal attention + output proj), tensor-parallel over 8 NeuronCores.

Sharding: heads are split across cores (4 Q heads + 1 KV head per core).
Each core computes its QKV shard for all tokens, runs attention for its
heads, then the per-core attention outputs (head-sharded) are AllGathered
(chunked over token groups, overlapped with compute) and each core computes
a 512-column slice of the output projection. All token-indexed tensors live
on-device in transposed layout ([feature, token]) so the hidden-dim
contraction lands on the partition axis for the TensorEngine; the host
transposes inputs/outputs during shard/unshard.
"""

from contextlib import ExitStack

import numpy as np
import ml_dtypes

import concourse.bass as bass
from concourse import bacc
import concourse.tile as tile
import concourse.mybir as mybir
from concourse.bass_utils import run_bass_kernel_spmd

F32 = mybir.dt.float32
F32R = mybir.dt.float32r
BF16 = mybir.dt.bfloat16
EXP = mybir.ActivationFunctionType.Exp
LN = mybir.ActivationFunctionType.Ln

N_CORES = 8
N_HEADS = 32
N_KV_HEADS = 8
D = 128          # head dim
HID = 4096
B = 2
S = 2048
T = B * S        # 4096 tokens
ROPE_BASE = 10000.0

HL = N_HEADS // N_CORES          # 4 local Q heads per core
QKV_ROWS = (HL + 2) * D          # 768: 4 Q heads + 1 K head + 1 V head
JC = HID // N_CORES              # 512 output columns per core

TC = 256                         # token chunk for the QKV projection phase
QC = 512                         # query chunk in attention / o_proj
N_HT = HID // 128                # 32 hidden tiles
AGC = 256                        # AllGather chunk granularity (tokens)
N_CH = T // AGC                  # 16 AllGather chunks


def _emit(tc_ctx, xt, wqkvt, wot, ropes, out_t, qkt, ag_ins, ag_outs):
    nc = tc_ctx.nc
    n_tc = T // TC
    n_kt = S // 128          # 16 k-tiles per batch
    n_qc = S // QC           # 4 q-chunks per batch

    with ExitStack() as es:
        const_pool = es.enter_context(tc_ctx.tile_pool(name="const", bufs=1))
        ones_init = const_pool.tile([128, 128], F32)
        nc.vector.memset(ones_init, 1.0)
        # All-ones stationary: one matmul computes column sums AND broadcasts
        # them across all 128 partitions.
        ones_mat = const_pool.tile([128, 128], F32R)
        nc.vector.tensor_copy(ones_mat, ones_init)
        # Diagonal causal masks: mask_d[k, q] = 1.0 if q - k - 128*d >= 0 else 0
        masks = const_pool.tile([128, 4, QC], BF16)
        nc.vector.memset(masks, 1.0)
        for d_off in range(4):
            nc.gpsimd.affine_select(
                out=masks[:, d_off, :],
                in_=masks[:, d_off, :],
                compare_op=mybir.AluOpType.is_ge,
                fill=0.0,
                base=-128 * d_off,
                pattern=[[1, QC]],
                channel_multiplier=-1,
            )

        # Pools for attention inputs, opened early so batch-0 K/V/Q loads can
        # overlap the tail of phase 1.
        qpool = es.enter_context(tc_ctx.tile_pool(name="p2_q", bufs=2))
        kvpool = es.enter_context(tc_ctx.tile_pool(name="p2_kv", bufs=2))
        kvq = {}

        def load_kvq(b):
            k_sb = kvpool.tile([128, S], BF16, tag="k", name=f"k_sb{b}")
            nc.sync.dma_start(
                out=k_sb, in_=qkt[HL * 128:(HL + 1) * 128, b * S:(b + 1) * S]
            )
            v_sb = kvpool.tile([128, n_kt, 128], BF16, tag="v", name=f"v_sb{b}")
            for kt in range(n_kt):
                nc.scalar.dma_start(
                    out=v_sb[:, kt, :],
                    in_=qkt[(HL + 1) * 128:(HL + 2) * 128,
                            b * S + kt * 128:b * S + (kt + 1) * 128],
                    transpose=True,
                )
            q_sb = qpool.tile([128, HL, S], BF16, name=f"q_sb{b}")
            nc.sync.dma_start(
                out=q_sb,
                in_=qkt.rearrange("(hh p) t -> p hh t", p=128)[
                    :, 0:HL, b * S:(b + 1) * S
                ],
            )
            kvq[b] = (k_sb, v_sb, q_sb)

        # ---------------- Phase 1: QKV projection + RoPE --------------------
        with tc_ctx.tile_pool(name="p1_w", bufs=1) as wpool, \
             tc_ctx.tile_pool(name="p1_x", bufs=3) as xpool, \
             tc_ctx.tile_pool(name="p1_rope", bufs=2) as rpool, \
             tc_ctx.tile_pool(name="p1_ps", bufs=4, space="PSUM") as pspool, \
             tc_ctx.tile_pool(name="p1_out", bufs=4) as opool, \
             tc_ctx.tile_pool(name="p1_sh", bufs=4) as shpool:
            wq_sb = wpool.tile([128, HL + 2, N_HT, 128], BF16)
            for ot in range(HL + 2):
                # Weights ride the scalar-engine HWDGE queue so the first X
                # chunk (sync queue) lands in parallel.
                nc.scalar.dma_start(out=wq_sb[:, ot], in_=wqkvt.ap()[:, ot])
            for ch in range(n_tc):
                xt_sb = xpool.tile([128, N_HT, TC], BF16)
                for hq in range(2):
                    nc.sync.dma_start(
                        out=xt_sb[:, hq * 16:(hq + 1) * 16, :],
                        in_=xt.ap()[:, ch, hq * 16:(hq + 1) * 16, :],
                    )
                rope_sb = rpool.tile([128, 4, TC], F32)
                nc.sync.dma_start(
                    out=rope_sb,
                    in_=ropes.ap()[:, ch],
                )
                for ot in range(HL + 2):
                    ps = pspool.tile([128, TC], F32)
                    for h in range(N_HT):
                        nc.tensor.matmul(
                            ps,
                            lhsT=wq_sb[:, ot, h, :],
                            rhs=xt_sb[:, h, :],
                            start=(h == 0),
                            stop=(h == N_HT - 1),
                        )
                    if ot < HL + 1:
                        # RoPE for Q (ot<HL) and K (ot==HL).
                        cos_i = 0 if ot < HL else 2
                        sh = shpool.tile([128, TC], F32, tag="sh")
                        nc.vector.tensor_copy(sh[0:64, :], ps[64:128, :])
                        nc.vector.tensor_copy(sh[64:128, :], ps[0:64, :])
                        nc.vector.tensor_mul(sh, sh, rope_sb[:, cos_i + 1, :])
                        tmp = shpool.tile([128, TC], F32, tag="tmp")
                        nc.vector.tensor_mul(tmp, ps, rope_sb[:, cos_i, :])
                        qk_out = opool.tile([128, TC], BF16, tag="qk")
                        nc.vector.tensor_add(qk_out, tmp, sh)
                        nc.sync.dma_start(
                            out=qkt[ot * 128:(ot + 1) * 128,
                                    ch * TC:(ch + 1) * TC],
                            in_=qk_out,
                        )
                    else:
                        # V head: store [d, t] as-is; transposed on load later.
                        v_out = opool.tile([128, TC], BF16, tag="v")
                        nc.vector.tensor_copy(v_out, ps)
                        nc.sync.dma_start(
                            out=qkt[ot * 128:(ot + 1) * 128,
                                    ch * TC:(ch + 1) * TC],
                            in_=v_out,
                        )
                if ch == (S // TC) - 1:
                    # Batch-0 rows of qkt are complete: prefetch its K/V/Q now
                    # so attention can start the moment phase 1 drains.
                    load_kvq(0)

        # -------- Phases 2-4: attention + chunked AllGather + out-proj ------
        with tc_ctx.tile_pool(name="p2_p", bufs=4) as ppool, \
             tc_ctx.tile_pool(name="p2_cs", bufs=2) as cspool, \
             tc_ctx.tile_pool(name="p2_ps_s", bufs=3, space="PSUM") as ps_s, \
             tc_ctx.tile_pool(name="p2_ps_o", bufs=2, space="PSUM") as ps_o, \
             tc_ctx.tile_pool(name="p2_ps_b", bufs=2, space="PSUM") as ps_b, \
             tc_ctx.tile_pool(name="p2_misc", bufs=2) as mpool, \
             tc_ctx.tile_pool(name="p4_w", bufs=1) as wopool, \
             tc_ctx.tile_pool(name="p4_a", bufs=2) as apool, \
             tc_ctx.tile_pool(name="p4_ps", bufs=1, space="PSUM") as pspool4:
            wo_sb = wopool.tile([128, N_HT, JC], BF16)
            nc.sync.dma_start(out=wo_sb, in_=wot.ap())

            def emit_oproj(k):
                t0 = k * AGC
                ag_sb = apool.tile([128, N_HT, AGC], BF16)
                nc.sync.dma_start(
                    out=ag_sb,
                    in_=ag_outs[k].rearrange("(ht p) t -> p ht t", p=128),
                )
                for jt in range(JC // 128):
                    ps4 = pspool4.tile([128, AGC], F32)
                    for h in range(N_HT):
                        nc.tensor.matmul(
                            ps4,
                            lhsT=wo_sb[:, h, jt * 128:(jt + 1) * 128],
                            rhs=ag_sb[:, h, :],
                            start=(h == 0),
                            stop=(h == N_HT - 1),
                        )
                    res4 = apool.tile([128, AGC], F32, tag="res4")
                    nc.vector.tensor_copy(res4, ps4)
                    nc.sync.dma_start(
                        out=out_t[jt * 128:(jt + 1) * 128, t0:t0 + AGC],
                        in_=res4,
                    )

            ags_fired = []
            for b in range(B):
                if b not in kvq:
                    load_kvq(b)
            for qc in reversed(range(n_qc)):
                for b in range(B):
                    k_sb, v_sb, q_sb = kvq[b]
                    chunk = b * n_qc + qc
                    kt_max = (QC // 128) * qc + (QC // 128)
                    for hh in range(HL):
                        pso = ps_o.tile([128, QC], F32)
                        colsum = cspool.tile([128, QC], F32R, tag="colsum")
                        for kt in range(kt_max):
                            ps = ps_s.tile([128, QC], F32)
                            nc.tensor.matmul(
                                ps,
                                lhsT=k_sb[:, kt * 128:(kt + 1) * 128],
                                rhs=q_sb[:, hh, qc * QC:(qc + 1) * QC],
                                start=True,
                                stop=True,
                            )
                            pt = ppool.tile([128, QC], BF16)
                            nc.scalar.activation(pt, ps, EXP)
                            d_off = kt - (QC // 128) * qc
                            if d_off >= 0:
                                nc.vector.tensor_mul(pt, pt, masks[:, d_off, :])
                            nc.tensor.matmul(
                                pso,
                                lhsT=v_sb[:, kt, :],
                                rhs=pt,
                                start=(kt == 0),
                                stop=(kt == kt_max - 1),
                            )
                            if kt == 0:
                                nc.vector.tensor_copy(colsum, pt)
                            else:
                                nc.vector.tensor_add(colsum, colsum, pt)
                        # One matmul against an all-ones stationary both sums
                        # over k and broadcasts the sums to all partitions.
                        sums_bc = ps_b.tile([128, QC], F32)
                        nc.tensor.matmul(
                            sums_bc,
                            lhsT=ones_mat,
                            rhs=colsum,
                            start=True,
                            stop=True,
                        )
                        recip_bc = mpool.tile([128, QC], F32, tag="recip_bc")
                        rscr = mpool.tile([128, QC], F32, tag="rscr")
                        nc.vector.reciprocal_approx_accurate(
                            recip_bc, sums_bc, rscr
                        )
                        attn = mpool.tile([128, QC], BF16, tag="attn")
                        nc.vector.tensor_mul(attn, pso, recip_bc)
                        for sub in range(2):
                            nc.sync.dma_start(
                                out=ag_ins[2 * chunk + sub][
                                    hh * 128:(hh + 1) * 128, :],
                                in_=attn[:, sub * AGC:(sub + 1) * AGC],
                            )
                    for sub in range(2):
                        ck = 2 * chunk + sub
                        nc.gpsimd.collective_compute(
                            "AllGather",
                            mybir.AluOpType.bypass,
                            replica_groups=[list(range(N_CORES))],
                            ins=[ag_ins[ck][:]],
                            outs=[ag_outs[ck][:]],
                        )
                        # Output projection two AG chunks behind.
                        ags_fired.append(ck)
                        if len(ags_fired) >= 3:
                            emit_oproj(ags_fired[-3])
            emit_oproj(ags_fired[-2])
            emit_oproj(ags_fired[-1])


def _build_program():
    nc = bacc.Bacc("TRN2", target_bir_lowering=False, debug=False,
                   num_devices=N_CORES)
    xt = nc.declare_dram_parameter("xt", [128, T // TC, N_HT, TC], BF16,
                                   isOutput=False)
    wqkvt = nc.declare_dram_parameter("wqkvt", [128, HL + 2, N_HT, 128], BF16,
                                      isOutput=False)
    wot = nc.declare_dram_parameter("wot", [128, N_HT, JC], BF16,
                                    isOutput=False)
    ropes = nc.declare_dram_parameter("ropes", [128, T // TC, 4, TC], F32,
                                      isOutput=False)
    out_t = nc.declare_dram_parameter("out_t", [JC, T], F32, isOutput=True)

    qkt = nc.dram_tensor("qkt", [QKV_ROWS, T], BF16).ap()
    ag_ins = [nc.dram_tensor(f"ag_in{k}", [HL * D, AGC], BF16).ap()
              for k in range(N_CH)]
    ag_outs = [nc.dram_tensor(f"ag_out{k}", [N_HEADS * D, AGC], BF16,
                              addr_space="Shared").ap()
               for k in range(N_CH)]

    with tile.TileContext(nc) as tc_ctx:
        _emit(tc_ctx, xt, wqkvt, wot, ropes, out_t, qkt, ag_ins, ag_outs)
    nc.finalize()
    return nc


def _host_inputs(hidden_states, w_qkv, w_o):
    """Shard + transpose inputs for the 8 cores; returns in_maps."""
    X = np.asarray(hidden_states, dtype=np.float32).reshape(T, HID)
    # [p, ch, ht, tc] tiled layout so every DMA line is contiguous.
    xt = np.ascontiguousarray(
        X.reshape(T // TC, TC, N_HT, 128).transpose(3, 0, 2, 1)
    ).astype(ml_dtypes.bfloat16)

    # RoPE tables in [d, t] layout with rotate-half sign folded into sin and
    # the attention scale folded into the Q tables.
    inv_freq = 1.0 / (ROPE_BASE ** (np.arange(0, D, 2, dtype=np.float32) / D))
    pos = np.arange(S, dtype=np.float32)
    freqs = np.outer(pos, inv_freq)                      # (S, D/2)
    emb = np.concatenate([freqs, freqs], axis=-1)        # (S, D)
    cos = np.cos(emb).T.astype(np.float32)               # (D, S)
    sin = np.sin(emb).T.astype(np.float32)
    sgn = np.concatenate([-np.ones(D // 2), np.ones(D // 2)]).astype(np.float32)
    sins = sgn[:, None] * sin
    cos_t = np.tile(cos, (1, B))                         # (D, T)
    sins_t = np.tile(sins, (1, B))
    scale = np.float32(D ** -0.5)
    ropes = np.stack([cos_t * scale, sins_t * scale, cos_t, sins_t], axis=0)
    ropes = np.ascontiguousarray(
        ropes.reshape(4, 128, T // TC, TC).transpose(1, 2, 0, 3),
        dtype=np.float32,
    )

    w_qkv = np.asarray(w_qkv, dtype=np.float32)
    w_o = np.asarray(w_o, dtype=np.float32)
    q_sz = N_HEADS * D
    kv_sz = N_KV_HEADS * D
    in_maps = []
    for c in range(N_CORES):
        qr = w_qkv[c * HL * D:(c + 1) * HL * D]
        kr = w_qkv[q_sz + c * D:q_sz + (c + 1) * D]
        vr = w_qkv[q_sz + kv_sz + c * D:q_sz + kv_sz + (c + 1) * D]
        w_shard = np.concatenate([qr, kr, vr], axis=0)           # (768, HID)
        wqkvt_c = np.ascontiguousarray(
            w_shard.reshape(HL + 2, 128, N_HT, 128).transpose(3, 0, 2, 1)
        ).astype(ml_dtypes.bfloat16)
        wot_c = np.ascontiguousarray(
            w_o[c * JC:(c + 1) * JC, :].reshape(JC, N_HT, 128).transpose(2, 1, 0)
        ).astype(ml_dtypes.bfloat16)
        in_maps.append({
            "xt": xt, "wqkvt": wqkvt_c, "wot": wot_c, "ropes": ropes,
        })
    return in_maps


def _run(hidden_states, w_qkv, w_o, trace=False, tmpdir=None):
    in_maps = _host_inputs(hidden_states, w_qkv, w_o)
    nc = _build_program()
    res = run_bass_kernel_spmd(nc, in_maps, list(range(N_CORES)),
                               trace=trace, tmpdir=tmpdir)
    out_T = np.concatenate(
        [np.asarray(res.results[c]["out_t"]) for c in range(N_CORES)], axis=0
    )                                                     # (HID j, T)
    out = np.ascontiguousarray(out_T.T).reshape(B, S, HID).astype(np.float32)
    return out, res


def kernel(hidden_states, w_qkv, w_o):
    out, _ = _run(hidden_states, w_qkv, w_o, trace=False)
    return out


# revision 19
# speedup vs baseline: 1.0132x; 1.0132x over previous
"""Trainium2 Bass kernel: dense transformer attention block (QKV proj + RoPE +
GQA causal attention + output proj), tensor-parallel over 8 NeuronCores.

Sharding: heads are split across cores (4 Q heads + 1 KV head per core).
Each core computes its QKV shard for all tokens, runs attention for its
heads, then the per-core attention outputs (head-sharded) are AllGathered
(chunked over token groups, overlapped with compute) and each core computes
a 512-column slice of the output projection. All token-indexed tensors live
on-device in transposed layout ([feature, token]) so the hidden-dim
contraction lands on the partition axis for the TensorEngine; the host
transposes inputs/outputs during shard/unshard.
"""

from contextlib import ExitStack

import numpy as np
import ml_dtypes

import concourse.bass as bass
from concourse import bacc
import concourse.tile as tile
import concourse.mybir as mybir
from concourse.bass_utils import run_bass_kernel_spmd

F32 = mybir.dt.float32
F32R = mybir.dt.float32r
BF16 = mybir.dt.bfloat16
EXP = mybir.ActivationFunctionType.Exp
LN = mybir.ActivationFunctionType.Ln

N_CORES = 8
N_HEADS = 32
N_KV_HEADS = 8
D = 128          # head dim
HID = 4096
B = 2
S = 2048
T = B * S        # 4096 tokens
ROPE_BASE = 10000.0

HL = N_HEADS // N_CORES          # 4 local Q heads per core
QKV_ROWS = (HL + 2) * D          # 768: 4 Q heads + 1 K head + 1 V head
JC = HID // N_CORES              # 512 output columns per core

TC = 256                         # token chunk for the QKV projection phase
QC = 512                         # query chunk in attention / o_proj
N_HT = HID // 128                # 32 hidden tiles
AGC = 256                        # AllGather chunk granularity (tokens)
N_CH = T // AGC                  # 16 AllGather chunks


def _emit(tc_ctx, xt, wqkvt, wot, ropes, out_t, qkt, ag_ins, ag_outs):
    nc = tc_ctx.nc
    n_tc = T // TC
    n_kt = S // 128          # 16 k-tiles per batch
    n_qc = S // QC           # 4 q-chunks per batch

    with ExitStack() as es:
        const_pool = es.enter_context(tc_ctx.tile_pool(name="const", bufs=1))
        ones_init = const_pool.tile([128, 128], F32)
        nc.vector.memset(ones_init, 1.0)
        # All-ones stationary: one matmul computes column sums AND broadcasts
        # them across all 128 partitions.
        ones_mat = const_pool.tile([128, 128], F32R)
        nc.vector.tensor_copy(ones_mat, ones_init)
        # Diagonal causal masks: mask_d[k, q] = 1.0 if q - k - 128*d >= 0 else 0
        masks = const_pool.tile([128, 4, QC], BF16)
        nc.vector.memset(masks, 1.0)
        for d_off in range(4):
            nc.gpsimd.affine_select(
                out=masks[:, d_off, :],
                in_=masks[:, d_off, :],
                compare_op=mybir.AluOpType.is_ge,
                fill=0.0,
                base=-128 * d_off,
                pattern=[[1, QC]],
                channel_multiplier=-1,
            )

        # Pools for attention inputs, opened early so batch-0 K/V/Q loads can
        # overlap the tail of phase 1.
        qpool = es.enter_context(tc_ctx.tile_pool(name="p2_q", bufs=2))
        kvpool = es.enter_context(tc_ctx.tile_pool(name="p2_kv", bufs=2))
        kvq = {}

        def load_kvq(b):
            k_sb = kvpool.tile([128, S], BF16, tag="k", name=f"k_sb{b}")
            nc.sync.dma_start(
                out=k_sb, in_=qkt[HL * 128:(HL + 1) * 128, b * S:(b + 1) * S]
            )
            v_sb = kvpool.tile([128, n_kt, 128], BF16, tag="v", name=f"v_sb{b}")
            for kt in range(n_kt):
                nc.scalar.dma_start(
                    out=v_sb[:, kt, :],
                    in_=qkt[(HL + 1) * 128:(HL + 2) * 128,
                            b * S + kt * 128:b * S + (kt + 1) * 128],
                    transpose=True,
                )
            q_sb = qpool.tile([128, HL, S], BF16, name=f"q_sb{b}")
            nc.sync.dma_start(
                out=q_sb,
                in_=qkt.rearrange("(hh p) t -> p hh t", p=128)[
                    :, 0:HL, b * S:(b + 1) * S
                ],
            )
            kvq[b] = (k_sb, v_sb, q_sb)

        # ---------------- Phase 1: QKV projection + RoPE --------------------
        with tc_ctx.tile_pool(name="p1_w", bufs=1) as wpool, \
             tc_ctx.tile_pool(name="p1_x", bufs=3) as xpool, \
             tc_ctx.tile_pool(name="p1_rope", bufs=2) as rpool, \
             tc_ctx.tile_pool(name="p1_ps", bufs=4, space="PSUM") as pspool, \
             tc_ctx.tile_pool(name="p1_out", bufs=4) as opool, \
             tc_ctx.tile_pool(name="p1_sh", bufs=4) as shpool:
            wq_sb = wpool.tile([128, HL + 2, N_HT, 128], BF16)
            for ot in range(HL + 2):
                # Weights ride the scalar-engine HWDGE queue so the first X
                # chunk (sync queue) lands in parallel.
                nc.scalar.dma_start(out=wq_sb[:, ot], in_=wqkvt.ap()[:, ot])
            for ch in range(n_tc):
                xt_sb = xpool.tile([128, N_HT, TC], BF16)
                for hq in range(2):
                    nc.sync.dma_start(
                        out=xt_sb[:, hq * 16:(hq + 1) * 16, :],
                        in_=xt.ap()[:, ch, hq * 16:(hq + 1) * 16, :],
                    )
                rope_sb = rpool.tile([128, 4, TC], F32)
                nc.sync.dma_start(
                    out=rope_sb,
                    in_=ropes.ap()[:, ch],
                )
                for ot in range(HL + 2):
                    ps = pspool.tile([128, TC], F32)
                    for h in range(N_HT):
                        nc.tensor.matmul(
                            ps,
                            lhsT=wq_sb[:, ot, h, :],
                            rhs=xt_sb[:, h, :],
                            start=(h == 0),
                            stop=(h == N_HT - 1),
                        )
                    if ot < HL + 1:
                        # RoPE for Q (ot<HL) and K (ot==HL).
                        cos_i = 0 if ot < HL else 2
                        sh = shpool.tile([128, TC], F32, tag="sh")
                        nc.vector.tensor_copy(sh[0:64, :], ps[64:128, :])
                        nc.vector.tensor_copy(sh[64:128, :], ps[0:64, :])
                        nc.vector.tensor_mul(sh, sh, rope_sb[:, cos_i + 1, :])
                        tmp = shpool.tile([128, TC], F32, tag="tmp")
                        nc.vector.tensor_mul(tmp, ps, rope_sb[:, cos_i, :])
                        qk_out = opool.tile([128, TC], BF16, tag="qk")
                        nc.vector.tensor_add(qk_out, tmp, sh)
                        nc.sync.dma_start(
                            out=qkt[ot * 128:(ot + 1) * 128,
                                    ch * TC:(ch + 1) * TC],
                            in_=qk_out,
                        )
                    else:
                        # V head: store [d, t] as-is; transposed on load later.
                        v_out = opool.tile([128, TC], BF16, tag="v")
                        nc.vector.tensor_copy(v_out, ps)
                        nc.sync.dma_start(
                            out=qkt[ot * 128:(ot + 1) * 128,
                                    ch * TC:(ch + 1) * TC],
                            in_=v_out,
                        )
                if ch == (S // TC) - 1:
                    # Batch-0 rows of qkt are complete: prefetch its K/V/Q now
                    # so attention can start the moment phase 1 drains.
                    load_kvq(0)

        # -------- Phases 2-4: attention + chunked AllGather + out-proj ------
        with tc_ctx.tile_pool(name="p2_p", bufs=4) as ppool, \
             tc_ctx.tile_pool(name="p2_cs", bufs=2) as cspool, \
             tc_ctx.tile_pool(name="p2_ps_s", bufs=3, space="PSUM") as ps_s, \
             tc_ctx.tile_pool(name="p2_ps_o", bufs=2, space="PSUM") as ps_o, \
             tc_ctx.tile_pool(name="p2_ps_b", bufs=2, space="PSUM") as ps_b, \
             tc_ctx.tile_pool(name="p2_misc", bufs=2) as mpool, \
             tc_ctx.tile_pool(name="p4_w", bufs=1) as wopool, \
             tc_ctx.tile_pool(name="p4_a", bufs=2) as apool, \
             tc_ctx.tile_pool(name="p4_ps", bufs=1, space="PSUM") as pspool4:
            wo_sb = wopool.tile([128, N_HT, JC], BF16)
            nc.sync.dma_start(out=wo_sb, in_=wot.ap())

            def emit_oproj(k):
                t0 = k * AGC
                ag_sb = apool.tile([128, N_HT, AGC], BF16)
                nc.sync.dma_start(
                    out=ag_sb,
                    in_=ag_outs[k].rearrange("(ht p) t -> p ht t", p=128),
                )
                for jt in range(JC // 128):
                    ps4 = pspool4.tile([128, AGC], F32)
                    for h in range(N_HT):
                        nc.tensor.matmul(
                            ps4,
                            lhsT=wo_sb[:, h, jt * 128:(jt + 1) * 128],
                            rhs=ag_sb[:, h, :],
                            start=(h == 0),
                            stop=(h == N_HT - 1),
                        )
                    res4 = apool.tile([128, AGC], F32, tag="res4")
                    nc.vector.tensor_copy(res4, ps4)
                    nc.sync.dma_start(
                        out=out_t[jt * 128:(jt + 1) * 128, t0:t0 + AGC],
                        in_=res4,
                    )

            ags_fired = []
            for b in range(B):
                if b not in kvq:
                    load_kvq(b)
            for qc in reversed(range(n_qc)):
                for b in range(B):
                    k_sb, v_sb, q_sb = kvq[b]
                    chunk = b * n_qc + qc
                    kt_max = (QC // 128) * qc + (QC // 128)
                    for hh in range(HL):
                        pso = ps_o.tile([128, QC], F32)
                        colsum = cspool.tile([128, QC], F32R, tag="colsum")
                        for kt in range(kt_max):
                            ps = ps_s.tile([128, QC], F32)
                            nc.tensor.matmul(
                                ps,
                                lhsT=k_sb[:, kt * 128:(kt + 1) * 128],
                                rhs=q_sb[:, hh, qc * QC:(qc + 1) * QC],
                                start=True,
                                stop=True,
                            )
                            pt = ppool.tile([128, QC], BF16)
                            nc.scalar.activation(pt, ps, EXP)
                            d_off = kt - (QC // 128) * qc
                            if d_off >= 0:
                                nc.vector.tensor_mul(pt, pt, masks[:, d_off, :])
                            nc.tensor.matmul(
                                pso,
                                lhsT=v_sb[:, kt, :],
                                rhs=pt,
                                start=(kt == 0),
                                stop=(kt == kt_max - 1),
                            )
                            if kt == 0:
                                nc.vector.tensor_copy(colsum, pt)
                            else:
                                nc.vector.tensor_add(colsum, colsum, pt)
                        # One matmul against an all-ones stationary both sums
                        # over k and broadcasts the sums to all partitions.
                        sums_bc = ps_b.tile([128, QC], F32)
                        nc.tensor.matmul(
                            sums_bc,
                            lhsT=ones_mat,
                            rhs=colsum,
                            start=True,
                            stop=True,
                        )
                        recip_bc = mpool.tile([128, QC], F32, tag="recip_bc")
                        rscr = mpool.tile([128, QC], F32, tag="rscr")
                        nc.vector.reciprocal_approx_accurate(
                            recip_bc, sums_bc, rscr
                        )
                        attn = mpool.tile([128, QC], BF16, tag="attn")
                        nc.vector.tensor_mul(attn, pso, recip_bc)
                        for sub in range(2):
                            nc.sync.dma_start(
                                out=ag_ins[2 * chunk + sub][
                                    hh * 128:(hh + 1) * 128, :],
                                in_=attn[:, sub * AGC:(sub + 1) * AGC],
                            )
                    for sub in range(2):
                        ck = 2 * chunk + sub
                        nc.gpsimd.collective_compute(
                            "AllGather",
                            mybir.AluOpType.bypass,
                            replica_groups=[list(range(N_CORES))],
                            ins=[ag_ins[ck][:]],
                            outs=[ag_outs[ck][:]],
                        )
                        # Output projection two AG chunks behind.
                        ags_fired.append(ck)
                        if len(ags_fired) >= 4:
                            emit_oproj(ags_fired[-4])
            for k in ags_fired[-3:]:
                emit_oproj(k)


def _build_program():
    nc = bacc.Bacc("TRN2", target_bir_lowering=False, debug=False,
                   num_devices=N_CORES)
    xt = nc.declare_dram_parameter("xt", [128, T // TC, N_HT, TC], BF16,
                                   isOutput=False)
    wqkvt = nc.declare_dram_parameter("wqkvt", [128, HL + 2, N_HT, 128], BF16,
                                      isOutput=False)
    wot = nc.declare_dram_parameter("wot", [128, N_HT, JC], BF16,
                                    isOutput=False)
    ropes = nc.declare_dram_parameter("ropes", [128, T // TC, 4, TC], F32,
                                      isOutput=False)
    out_t = nc.declare_dram_parameter("out_t", [JC, T], F32, isOutput=True)

    qkt = nc.dram_tensor("qkt", [QKV_ROWS, T], BF16).ap()
    ag_ins = [nc.dram_tensor(f"ag_in{k}", [HL * D, AGC], BF16).ap()
              for k in range(N_CH)]
    ag_outs = [nc.dram_tensor(f"ag_out{k}", [N_HEADS * D, AGC], BF16,
                              addr_space="Shared").ap()
               for k in range(N_CH)]

    with tile.TileContext(nc) as tc_ctx:
        _emit(tc_ctx, xt, wqkvt, wot, ropes, out_t, qkt, ag_ins, ag_outs)
    nc.finalize()
    return nc


def _host_inputs(hidden_states, w_qkv, w_o):
    """Shard + transpose inputs for the 8 cores; returns in_maps."""
    X = np.asarray(hidden_states, dtype=np.float32).reshape(T, HID)
    # [p, ch, ht, tc] tiled layout so every DMA line is contiguous.
    xt = np.ascontiguousarray(
        X.reshape(T // TC, TC, N_HT, 128).transpose(3, 0, 2, 1)
    ).astype(ml_dtypes.bfloat16)

    # RoPE tables in [d, t] layout with rotate-half sign folded into sin and
    # the attention scale folded into the Q tables.
    inv_freq = 1.0 / (ROPE_BASE ** (np.arange(0, D, 2, dtype=np.float32) / D))
    pos = np.arange(S, dtype=np.float32)
    freqs = np.outer(pos, inv_freq)                      # (S, D/2)
    emb = np.concatenate([freqs, freqs], axis=-1)        # (S, D)
    cos = np.cos(emb).T.astype(np.float32)               # (D, S)
    sin = np.sin(emb).T.astype(np.float32)
    sgn = np.concatenate([-np.ones(D // 2), np.ones(D // 2)]).astype(np.float32)
    sins = sgn[:, None] * sin
    cos_t = np.tile(cos, (1, B))                         # (D, T)
    sins_t = np.tile(sins, (1, B))
    scale = np.float32(D ** -0.5)
    ropes = np.stack([cos_t * scale, sins_t * scale, cos_t, sins_t], axis=0)
    ropes = np.ascontiguousarray(
        ropes.reshape(4, 128, T // TC, TC).transpose(1, 2, 0, 3),
        dtype=np.float32,
    )

    w_qkv = np.asarray(w_qkv, dtype=np.float32)
    w_o = np.asarray(w_o, dtype=np.float32)
    q_sz = N_HEADS * D
    kv_sz = N_KV_HEADS * D
    in_maps = []
    for c in range(N_CORES):
        qr = w_qkv[c * HL * D:(c + 1) * HL * D]
        kr = w_qkv[q_sz + c * D:q_sz + (c + 1) * D]
        vr = w_qkv[q_sz + kv_sz + c * D:q_sz + kv_sz + (c + 1) * D]
        w_shard = np.concatenate([qr, kr, vr], axis=0)           # (768, HID)
        wqkvt_c = np.ascontiguousarray(
            w_shard.reshape(HL + 2, 128, N_HT, 128).transpose(3, 0, 2, 1)
        ).astype(ml_dtypes.bfloat16)
        wot_c = np.ascontiguousarray(
            w_o[c * JC:(c + 1) * JC, :].reshape(JC, N_HT, 128).transpose(2, 1, 0)
        ).astype(ml_dtypes.bfloat16)
        in_maps.append({
            "xt": xt, "wqkvt": wqkvt_c, "wot": wot_c, "ropes": ropes,
        })
    return in_maps


def _run(hidden_states, w_qkv, w_o, trace=False, tmpdir=None):
    in_maps = _host_inputs(hidden_states, w_qkv, w_o)
    nc = _build_program()
    res = run_bass_kernel_spmd(nc, in_maps, list(range(N_CORES)),
                               trace=trace, tmpdir=tmpdir)
    out_T = np.concatenate(
        [np.asarray(res.results[c]["out_t"]) for c in range(N_CORES)], axis=0
    )                                                     # (HID j, T)
    out = np.ascontiguousarray(out_T.T).reshape(B, S, HID).astype(np.float32)
    return out, res


def kernel(hidden_states, w_qkv, w_o):
    out, _ = _run(hidden_states, w_qkv, w_o, trace=False)
    return out


# revision 20
# speedup vs baseline: 1.0435x; 1.0299x over previous
"""Trainium2 Bass kernel: dense transformer attention block (QKV proj + RoPE +
GQA causal attention + output proj), tensor-parallel over 8 NeuronCores.

Sharding: heads are split across cores (4 Q heads + 1 KV head per core).
Each core computes its QKV shard for all tokens, runs attention for its
heads, then the per-core attention outputs (head-sharded) are AllGathered
(chunked over token groups, overlapped with compute) and each core computes
a 512-column slice of the output projection. All token-indexed tensors live
on-device in transposed layout ([feature, token]) so the hidden-dim
contraction lands on the partition axis for the TensorEngine; the host
transposes inputs/outputs during shard/unshard.
"""

from contextlib import ExitStack

import numpy as np
import ml_dtypes

import concourse.bass as bass
from concourse import bacc
import concourse.tile as tile
import concourse.mybir as mybir
from concourse.bass_utils import run_bass_kernel_spmd

F32 = mybir.dt.float32
F32R = mybir.dt.float32r
BF16 = mybir.dt.bfloat16
EXP = mybir.ActivationFunctionType.Exp
LN = mybir.ActivationFunctionType.Ln

N_CORES = 8
N_HEADS = 32
N_KV_HEADS = 8
D = 128          # head dim
HID = 4096
B = 2
S = 2048
T = B * S        # 4096 tokens
ROPE_BASE = 10000.0

HL = N_HEADS // N_CORES          # 4 local Q heads per core
QKV_ROWS = (HL + 2) * D          # 768: 4 Q heads + 1 K head + 1 V head
JC = HID // N_CORES              # 512 output columns per core

TC = 256                         # token chunk for the QKV projection phase
QC = 512                         # query chunk in attention / o_proj
N_HT = HID // 128                # 32 hidden tiles
AGC = 256                        # AllGather chunk granularity (tokens)
N_CH = T // AGC                  # 16 AllGather chunks


def _emit(tc_ctx, xt, wqkvt, wot, ropes, out_t, qkt, ag_ins, ag_outs):
    nc = tc_ctx.nc
    n_tc = T // TC
    n_kt = S // 128          # 16 k-tiles per batch
    n_qc = S // QC           # 4 q-chunks per batch

    with ExitStack() as es:
        const_pool = es.enter_context(tc_ctx.tile_pool(name="const", bufs=1))
        ones_init = const_pool.tile([128, 128], F32)
        nc.vector.memset(ones_init, 1.0)
        # All-ones stationary: one matmul computes column sums AND broadcasts
        # them across all 128 partitions.
        ones_mat = const_pool.tile([128, 128], F32R)
        nc.vector.tensor_copy(ones_mat, ones_init)
        # Diagonal causal masks: mask_d[k, q] = 1.0 if q - k - 128*d >= 0 else 0
        masks = const_pool.tile([128, 4, QC], BF16)
        nc.vector.memset(masks, 1.0)
        for d_off in range(4):
            nc.gpsimd.affine_select(
                out=masks[:, d_off, :],
                in_=masks[:, d_off, :],
                compare_op=mybir.AluOpType.is_ge,
                fill=0.0,
                base=-128 * d_off,
                pattern=[[1, QC]],
                channel_multiplier=-1,
            )

        # Pools for attention inputs, opened early so batch-0 K/V/Q loads can
        # overlap the tail of phase 1.
        qpool = es.enter_context(tc_ctx.tile_pool(name="p2_q", bufs=2))
        kvpool = es.enter_context(tc_ctx.tile_pool(name="p2_kv", bufs=2))
        kvq = {}

        def load_kvq(b):
            k_sb = kvpool.tile([128, S], BF16, tag="k", name=f"k_sb{b}")
            nc.sync.dma_start(
                out=k_sb, in_=qkt[HL * 128:(HL + 1) * 128, b * S:(b + 1) * S]
            )
            v_sb = kvpool.tile([128, n_kt, 128], BF16, tag="v", name=f"v_sb{b}")
            for kt in range(n_kt):
                nc.scalar.dma_start(
                    out=v_sb[:, kt, :],
                    in_=qkt[(HL + 1) * 128:(HL + 2) * 128,
                            b * S + kt * 128:b * S + (kt + 1) * 128],
                    transpose=True,
                )
            q_sb = qpool.tile([128, HL, S], BF16, name=f"q_sb{b}")
            nc.sync.dma_start(
                out=q_sb,
                in_=qkt.rearrange("(hh p) t -> p hh t", p=128)[
                    :, 0:HL, b * S:(b + 1) * S
                ],
            )
            kvq[b] = (k_sb, v_sb, q_sb)

        # ---------------- Phase 1: QKV projection + RoPE --------------------
        with tc_ctx.tile_pool(name="p1_w", bufs=1) as wpool, \
             tc_ctx.tile_pool(name="p1_x", bufs=3) as xpool, \
             tc_ctx.tile_pool(name="p1_rope", bufs=2) as rpool, \
             tc_ctx.tile_pool(name="p1_ps", bufs=4, space="PSUM") as pspool, \
             tc_ctx.tile_pool(name="p1_out", bufs=4) as opool, \
             tc_ctx.tile_pool(name="p1_sh", bufs=4) as shpool:
            wq_sb = wpool.tile([128, HL + 2, N_HT, 128], BF16)
            for ot in range(HL + 2):
                # Weights ride the scalar-engine HWDGE queue so the first X
                # chunk (sync queue) lands in parallel.
                nc.scalar.dma_start(out=wq_sb[:, ot], in_=wqkvt.ap()[:, ot])
            for ch in range(n_tc):
                xt_sb = xpool.tile([128, N_HT, TC], BF16)
                for hq in range(2):
                    nc.sync.dma_start(
                        out=xt_sb[:, hq * 16:(hq + 1) * 16, :],
                        in_=xt.ap()[:, ch, hq * 16:(hq + 1) * 16, :],
                    )
                rope_sb = rpool.tile([128, 4, TC], F32)
                nc.sync.dma_start(
                    out=rope_sb,
                    in_=ropes.ap()[:, ch],
                )
                for ot in range(HL + 2):
                    ps = pspool.tile([128, TC], F32)
                    for h in range(N_HT):
                        nc.tensor.matmul(
                            ps,
                            lhsT=wq_sb[:, ot, h, :],
                            rhs=xt_sb[:, h, :],
                            start=(h == 0),
                            stop=(h == N_HT - 1),
                        )
                    if ot < HL + 1:
                        # RoPE for Q (ot<HL) and K (ot==HL).
                        cos_i = 0 if ot < HL else 2
                        sh = shpool.tile([128, TC], F32, tag="sh")
                        nc.vector.tensor_copy(sh[0:64, :], ps[64:128, :])
                        nc.vector.tensor_copy(sh[64:128, :], ps[0:64, :])
                        nc.vector.tensor_mul(sh, sh, rope_sb[:, cos_i + 1, :])
                        tmp = shpool.tile([128, TC], F32, tag="tmp")
                        nc.vector.tensor_mul(tmp, ps, rope_sb[:, cos_i, :])
                        qk_out = opool.tile([128, TC], BF16, tag="qk")
                        nc.vector.tensor_add(qk_out, tmp, sh)
                        nc.sync.dma_start(
                            out=qkt[ot * 128:(ot + 1) * 128,
                                    ch * TC:(ch + 1) * TC],
                            in_=qk_out,
                        )
                    else:
                        # V head: store [d, t] as-is; transposed on load later.
                        v_out = opool.tile([128, TC], BF16, tag="v")
                        nc.vector.tensor_copy(v_out, ps)
                        nc.sync.dma_start(
                            out=qkt[ot * 128:(ot + 1) * 128,
                                    ch * TC:(ch + 1) * TC],
                            in_=v_out,
                        )
                if ch == (S // TC) - 1:
                    # Batch-0 rows of qkt are complete: prefetch its K/V/Q now
                    # so attention can start the moment phase 1 drains.
                    load_kvq(0)

        # -------- Phases 2-4: attention + chunked AllGather + out-proj ------
        with tc_ctx.tile_pool(name="p2_p", bufs=4) as ppool, \
             tc_ctx.tile_pool(name="p2_cs", bufs=2) as cspool, \
             tc_ctx.tile_pool(name="p2_ps_s", bufs=3, space="PSUM") as ps_s, \
             tc_ctx.tile_pool(name="p2_ps_o", bufs=2, space="PSUM") as ps_o, \
             tc_ctx.tile_pool(name="p2_ps_b", bufs=2, space="PSUM") as ps_b, \
             tc_ctx.tile_pool(name="p2_misc", bufs=2) as mpool, \
             tc_ctx.tile_pool(name="p4_w", bufs=1) as wopool, \
             tc_ctx.tile_pool(name="p4_a", bufs=2) as apool, \
             tc_ctx.tile_pool(name="p4_ps", bufs=1, space="PSUM") as pspool4:
            wo_sb = wopool.tile([128, N_HT, JC], BF16)
            nc.sync.dma_start(out=wo_sb, in_=wot.ap())

            def emit_oproj(k):
                t0 = k * AGC
                ag_sb = apool.tile([128, N_HT, AGC], BF16)
                nc.sync.dma_start(
                    out=ag_sb,
                    in_=ag_outs[k].rearrange("(ht p) t -> p ht t", p=128),
                )
                for jt in range(JC // 128):
                    ps4 = pspool4.tile([128, AGC], F32)
                    for h in range(N_HT):
                        nc.tensor.matmul(
                            ps4,
                            lhsT=wo_sb[:, h, jt * 128:(jt + 1) * 128],
                            rhs=ag_sb[:, h, :],
                            start=(h == 0),
                            stop=(h == N_HT - 1),
                        )
                    res4 = apool.tile([128, AGC], F32, tag="res4")
                    nc.vector.tensor_copy(res4, ps4)
                    nc.sync.dma_start(
                        out=out_t[jt * 128:(jt + 1) * 128, t0:t0 + AGC],
                        in_=res4,
                    )

            ags_fired = []
            for b in range(B):
                if b not in kvq:
                    load_kvq(b)
            for qc in reversed(range(n_qc)):
                for b in range(B):
                    k_sb, v_sb, q_sb = kvq[b]
                    chunk = b * n_qc + qc
                    kt_max = (QC // 128) * qc + (QC // 128)
                    for hh in range(HL):
                        pso = ps_o.tile([128, QC], F32)
                        colsum = cspool.tile([128, QC], F32R, tag="colsum")
                        for kt in range(kt_max):
                            ps = ps_s.tile([128, QC], F32)
                            nc.tensor.matmul(
                                ps,
                                lhsT=k_sb[:, kt * 128:(kt + 1) * 128],
                                rhs=q_sb[:, hh, qc * QC:(qc + 1) * QC],
                                start=True,
                                stop=True,
                            )
                            pt = ppool.tile([128, QC], BF16)
                            nc.scalar.activation(pt, ps, EXP)
                            d_off = kt - (QC // 128) * qc
                            if d_off >= 0:
                                nc.vector.tensor_mul(pt, pt, masks[:, d_off, :])
                            nc.tensor.matmul(
                                pso,
                                lhsT=v_sb[:, kt, :],
                                rhs=pt,
                                start=(kt == 0),
                                stop=(kt == kt_max - 1),
                            )
                            if kt == 0:
                                nc.vector.tensor_copy(colsum, pt)
                            else:
                                nc.vector.tensor_add(colsum, colsum, pt)
                        # One matmul against an all-ones stationary both sums
                        # over k and broadcasts the sums to all partitions.
                        sums_bc = ps_b.tile([128, QC], F32)
                        nc.tensor.matmul(
                            sums_bc,
                            lhsT=ones_mat,
                            rhs=colsum,
                            start=True,
                            stop=True,
                        )
                        recip_bc = mpool.tile([128, QC], F32, tag="recip_bc")
                        rscr = mpool.tile([128, QC], F32, tag="rscr")
                        nc.vector.reciprocal_approx_accurate(
                            recip_bc, sums_bc, rscr
                        )
                        attn = mpool.tile([128, QC], BF16, tag="attn")
                        nc.vector.tensor_mul(attn, pso, recip_bc)
                        for sub in range(2):
                            nc.sync.dma_start(
                                out=ag_ins[2 * chunk + sub][
                                    hh * 128:(hh + 1) * 128, :],
                                in_=attn[:, sub * AGC:(sub + 1) * AGC],
                            )
                    for sub in range(2):
                        ck = 2 * chunk + sub
                        nc.gpsimd.collective_compute(
                            "AllGather",
                            mybir.AluOpType.bypass,
                            replica_groups=[list(range(N_CORES))],
                            ins=[ag_ins[ck][:]],
                            outs=[ag_outs[ck][:]],
                        )
                        # Output projection two AG chunks behind.
                        ags_fired.append(ck)
                        if len(ags_fired) >= 5:
                            emit_oproj(ags_fired[-5])
            for k in ags_fired[-4:]:
                emit_oproj(k)


def _build_program():
    nc = bacc.Bacc("TRN2", target_bir_lowering=False, debug=False,
                   num_devices=N_CORES)
    xt = nc.declare_dram_parameter("xt", [128, T // TC, N_HT, TC], BF16,
                                   isOutput=False)
    wqkvt = nc.declare_dram_parameter("wqkvt", [128, HL + 2, N_HT, 128], BF16,
                                      isOutput=False)
    wot = nc.declare_dram_parameter("wot", [128, N_HT, JC], BF16,
                                    isOutput=False)
    ropes = nc.declare_dram_parameter("ropes", [128, T // TC, 4, TC], F32,
                                      isOutput=False)
    out_t = nc.declare_dram_parameter("out_t", [JC, T], F32, isOutput=True)

    qkt = nc.dram_tensor("qkt", [QKV_ROWS, T], BF16).ap()
    ag_ins = [nc.dram_tensor(f"ag_in{k}", [HL * D, AGC], BF16).ap()
              for k in range(N_CH)]
    ag_outs = [nc.dram_tensor(f"ag_out{k}", [N_HEADS * D, AGC], BF16,
                              addr_space="Shared").ap()
               for k in range(N_CH)]

    with tile.TileContext(nc) as tc_ctx:
        _emit(tc_ctx, xt, wqkvt, wot, ropes, out_t, qkt, ag_ins, ag_outs)
    nc.finalize()
    return nc


def _host_inputs(hidden_states, w_qkv, w_o):
    """Shard + transpose inputs for the 8 cores; returns in_maps."""
    X = np.asarray(hidden_states, dtype=np.float32).reshape(T, HID)
    # [p, ch, ht, tc] tiled layout so every DMA line is contiguous.
    xt = np.ascontiguousarray(
        X.reshape(T // TC, TC, N_HT, 128).transpose(3, 0, 2, 1)
    ).astype(ml_dtypes.bfloat16)

    # RoPE tables in [d, t] layout with rotate-half sign folded into sin and
    # the attention scale folded into the Q tables.
    inv_freq = 1.0 / (ROPE_BASE ** (np.arange(0, D, 2, dtype=np.float32) / D))
    pos = np.arange(S, dtype=np.float32)
    freqs = np.outer(pos, inv_freq)                      # (S, D/2)
    emb = np.concatenate([freqs, freqs], axis=-1)        # (S, D)
    cos = np.cos(emb).T.astype(np.float32)               # (D, S)
    sin = np.sin(emb).T.astype(np.float32)
    sgn = np.concatenate([-np.ones(D // 2), np.ones(D // 2)]).astype(np.float32)
    sins = sgn[:, None] * sin
    cos_t = np.tile(cos, (1, B))                         # (D, T)
    sins_t = np.tile(sins, (1, B))
    scale = np.float32(D ** -0.5)
    ropes = np.stack([cos_t * scale, sins_t * scale, cos_t, sins_t], axis=0)
    ropes = np.ascontiguousarray(
        ropes.reshape(4, 128, T // TC, TC).transpose(1, 2, 0, 3),
        dtype=np.float32,
    )

    w_qkv = np.asarray(w_qkv, dtype=np.float32)
    w_o = np.asarray(w_o, dtype=np.float32)
    q_sz = N_HEADS * D
    kv_sz = N_KV_HEADS * D
    in_maps = []
    for c in range(N_CORES):
        qr = w_qkv[c * HL * D:(c + 1) * HL * D]
        kr = w_qkv[q_sz + c * D:q_sz + (c + 1) * D]
        vr = w_qkv[q_sz + kv_sz + c * D:q_sz + kv_sz + (c + 1) * D]
        w_shard = np.concatenate([qr, kr, vr], axis=0)           # (768, HID)
        wqkvt_c = np.ascontiguousarray(
            w_shard.reshape(HL + 2, 128, N_HT, 128).transpose(3, 0, 2, 1)
        ).astype(ml_dtypes.bfloat16)
        wot_c = np.ascontiguousarray(
            w_o[c * JC:(c + 1) * JC, :].reshape(JC, N_HT, 128).transpose(2, 1, 0)
        ).astype(ml_dtypes.bfloat16)
        in_maps.append({
            "xt": xt, "wqkvt": wqkvt_c, "wot": wot_c, "ropes": ropes,
        })
    return in_maps


def _run(hidden_states, w_qkv, w_o, trace=False, tmpdir=None):
    in_maps = _host_inputs(hidden_states, w_qkv, w_o)
    nc = _build_program()
    res = run_bass_kernel_spmd(nc, in_maps, list(range(N_CORES)),
                               trace=trace, tmpdir=tmpdir)
    out_T = np.concatenate(
        [np.asarray(res.results[c]["out_t"]) for c in range(N_CORES)], axis=0
    )                                                     # (HID j, T)
    out = np.ascontiguousarray(out_T.T).reshape(B, S, HID).astype(np.float32)
    return out, res


def kernel(hidden_states, w_qkv, w_o):
    out, _ = _run(hidden_states, w_qkv, w_o, trace=False)
    return out
